# revision 1
# baseline (speedup 1.0000x reference)
"""Trainium2 Bass kernel for nn_CircularBlur: depthwise 4x4 blur with
circular padding on (4, 512, 256, 256) fp32.

Math (derived from the reference's wrap-pad + zero-pad + flipped-kernel
conv + crop; the zero padding never reaches the cropped region):

    out[n,c,y,x] = sum_{i,j} k[i,j] * in[n,c,(y+1-i)%256,(x+1-j)%256]

Strategy: pure data parallel over the 2048 (n,c) images, 256 per core.
Per image the blur is separable (k = a outer b via SVD).  The vertical
pass is a banded-circulant matmul on the tensor engine (stationary =
128x128 chunks of V^T, prescaled by the horizontal tap weights).  The
horizontal taps become shifted column windows of the moving operand;
symmetric tap pairs are pre-summed on the vector engine so each pair
costs one matmul instead of two.  Column wrap is handled with a 3-col
halo in the on-chip tile; row wrap is baked into V.

The rel-err budget (2e-2) is spent on fp16 I/O: the host ships x as
fp16 and reads y back as fp16 (~3e-4 quantization), cutting HBM
traffic from 128 to 64 MiB per core.  Engine balance:
  - all element-wise work runs on the Vector engine.  With full-256
    wide halo windows the fp16 adds hit the DVE 2-elem/cycle fast path
    (~1.1us per half-group); GpSimd is kept fully idle so DVE never
    loses the shared SBUF port-pair arbitration.
  - the tensor engine (fp16 matmuls + fast weight load, ~237us) is the
    expected bottleneck; loads, stores, copies and descriptor
    generation are spread across Sync/ACT/DVE so nothing else binds.
  - work is chunked at half-group granularity (4 images) with separate
    tiles per half: short ramp, small tail, smooth pipeline.
"""

import sys

sys.path.insert(0, "/opt/trn_rl_repo")

import numpy as np

N_CORES = 8
H = W = 256
IMG_TOTAL = 4 * 512
IMG_PER_CORE = IMG_TOTAL // N_CORES  # 256
G = 8   # images per group
HG = 4  # images per half-group (one DMA)
NGROUPS = IMG_PER_CORE // G
KH = KW = 4


def _decompose(k):
    """k (4,4) float64 -> list of rank-1 terms [(a[4], taps)], where
    taps = [(scale, [shifts...])]; shift s means column x+s contributes
    with weight scale (after pre-summing all shifts in the group)."""
    U, S, Vt = np.linalg.svd(k)
    terms = []
    for r in range(KH):
        if S[r] <= max(S[0] * 1e-7, 1e-30):
            continue
        a = U[:, r] * np.sqrt(S[r])
        b = Vt[r] * np.sqrt(S[r])
        # tap j has shift 1-j and weight b[j]
        tol = 1e-9 * max(1.0, np.abs(b).max())
        if abs(b[0] - b[3]) <= tol and abs(b[1] - b[2]) <= tol:
            taps = [(b[0], [1, -2]), (b[1], [0, -1])]
        else:
            taps = [(b[j], [1 - j]) for j in range(KW)]
        terms.append((a, taps))
    return terms


def _build_weights(terms):
    """Host-side stationary blocks.  Returns (W_host [128, NIDX, 128] f16,
    mov_shifts: list of shift-lists, one per moving tensor)."""
    movs = []  # (a_vec, scale, shifts)
    for a, taps in terms:
        for scale, shifts in taps:
            movs.append((a, scale, shifts))
    n_idx = len(movs) * 4
    Wh = np.zeros((128, n_idx, 128), np.float32)
    yy = np.arange(H)
    for mi, (a, scale, _shifts) in enumerate(movs):
        V = np.zeros((H, H), np.float64)
        for i in range(KH):
            V[yy, (yy + 1 - i) % H] += a[i]
        VT = (scale * V).T  # VT[v, y]
        for kc in range(2):
            for yb in range(2):
                idx = (mi * 2 + kc) * 2 + yb
                # row v=2*vp+kc lives on partition vp; out row y=2*m+yb on
                # psum partition m (even/odd interleave -> aligned DMA rows)
                Wh[:, idx, :] = VT[kc::2, yb::2].astype(np.float32)
    return Wh.astype(np.float16), [m[2] for m in movs]


_PROGRAM_CACHE = {}

# groups with g % MERGE_EVERY == 3 run the merged-mov path (when the
# decomposition allows it): M = r*u + t via one fused DVE op, then a
# single accumulation chain of 2 matmuls per psum instead of 4.  This
# trades ~3.6us/group of tensor-engine time (the bottleneck) for
# ~2.3us/group of vector-engine slack.
MERGE_EVERY = 8


def _build_program(mov_shifts, merge_ratio=None):
    """Build + compile the per-core Bass program.  mov_shifts: list of
    shift-lists (structure only); merge_ratio: scale ratio s1/s0 for the
    merged-mov groups, or None to disable merging.  Weights arrive via
    the `w` input."""
    import concourse.bacc as bacc
    import concourse.mybir as mybir
    from concourse import tile
    from concourse.alu_op_type import AluOpType

    key = (tuple(tuple(s) for s in mov_shifts),
           None if merge_ratio is None else float(np.float32(merge_ratio)))
    if key in _PROGRAM_CACHE:
        return _PROGRAM_CACHE[key]

    f16 = mybir.dt.float16
    f32 = mybir.dt.float32
    n_movs = len(mov_shifts)
    n_idx = n_movs * 4

    nc = bacc.Bacc("TRN2", target_bir_lowering=False, debug=False,
                   num_devices=N_CORES)
    x_in = nc.declare_dram_parameter("x", [IMG_PER_CORE, H, W], f16,
                                     isOutput=False)
    w_in = nc.declare_dram_parameter("w", [128, n_idx, 128], f16,
                                     isOutput=False)
    y_out = nc.declare_dram_parameter("y", [IMG_PER_CORE, H, W], f16,
                                      isOutput=True)

    # halo layout: xc col c holds image col x = c - 2 for c in [2, 258);
    # cols 0,1,258 are circular-wrap copies, so every shifted window
    # (shifts in [-2, +1]) is contiguous and full-width -- which is what
    # lets the DVE adds run in the 2-elem/cycle fast path
    HW_ = W + 3

    def fill_halo(xc):
        for c, src in ((0, 256), (1, 257), (258, 2)):
            nc.vector.tensor_copy(xc[:, :, :, c:c + 1],
                                  xc[:, :, :, src:src + 1])

    def build_mov(pt, xc, shifts):
        """pt[x] = sum_s xc[(x+s) % W], one full-width op (halo covers
        the wrap)."""
        if len(shifts) == 1:
            s = shifts[0] + 2
            nc.vector.tensor_copy(pt[:], xc[:, :, :, s:s + W])
        else:
            assert len(shifts) == 2
            s0, s1 = shifts[0] + 2, shifts[1] + 2
            nc.vector.tensor_add(
                pt[:], xc[:, :, :, s0:s0 + W], xc[:, :, :, s1:s1 + W]
            )

    with tile.TileContext(nc) as tc:
        with (
            tc.tile_pool(name="const", bufs=1) as cpool,
            tc.tile_pool(name="xin", bufs=6) as xpool,
            tc.tile_pool(name="mov", bufs=3) as mpool,
            tc.tile_pool(name="outp", bufs=4) as opool,
            tc.tile_pool(name="psum", bufs=6, space="PSUM") as pspool,
        ):
            # weight load on the ACT (scalar) queue so the sync queue's
            # first x-load descriptor generation starts immediately; one
            # DMA per idx block so the per-partition descriptors spread
            # over all 16 SDMA engines instead of landing on one
            wt = cpool.tile([128, n_idx, 128], f16)
            for i in range(n_idx):
                nc.scalar.dma_start(wt[:, i, :], w_in[:, i, :])

            for g in range(NGROUPS):
                merged = merge_ratio is not None and g % MERGE_EVERY == 3
                movs = [[None] * 2 for _ in range(n_movs)]
                mrg = [None, None]
                yts = [None, None]
                for h in range(2):
                    hs = slice(g * G + h * HG, g * G + (h + 1) * HG)
                    xc = xpool.tile([128, HG, 2, HW_], f16, tag=f"xc{h}",
                                    name=f"xc{h}")
                    # one DMA per row-parity: the padded (stride-259)
                    # tile only AP-balances as [128][img][256] per r
                    # slice; the finer granularity also ramps the SDMA
                    # engines quickly at the head
                    src = x_in[hs].rearrange("m (p r) w -> p m r w", r=2)
                    for rr in range(2):
                        nc.sync.dma_start(
                            xc[:, :, rr, 2:2 + W], src[:, :, rr, :]
                        )
                    fill_halo(xc)
                    for ti, shifts in enumerate(mov_shifts):
                        pt = mpool.tile([128, HG, 2, W], f16,
                                        tag=f"p{ti}h{h}", name=f"p{ti}h{h}")
                        build_mov(pt, xc, shifts)
                        movs[ti][h] = pt
                    if merged:
                        mm_t = mpool.tile([128, HG, 2, W], f16,
                                          tag=f"mg{h}", name=f"mg{h}",
                                          bufs=1)
                        nc.vector.scalar_tensor_tensor(
                            mm_t[:], movs[1][h][:], merge_ratio,
                            movs[0][h][:], AluOpType.mult, AluOpType.add,
                        )
                        mrg[h] = mm_t
                    yts[h] = opool.tile([128, HG, 2, W], f16, tag=f"yt{h}",
                                        name=f"yt{h}")

                for pr in range(G // 2):
                    h, j = pr // 2, (pr % 2) * 2
                    yt = yts[h]
                    for yb in range(2):
                        ps = pspool.tile([128, 2, W], f32, tag="ps")
                        if merged:
                            mms = [(0, kc) for kc in range(2)]
                            src = mrg
                        else:
                            mms = [(mi, kc) for mi in range(n_movs)
                                   for kc in range(2)]
                            src = None
                        for q, (mi, kc) in enumerate(mms):
                            idx = (mi * 2 + kc) * 2 + yb
                            base = src[h] if merged else movs[mi][h]
                            rhs = base[:, j:j + 2, kc, 0:W]
                            nc.tensor.matmul(
                                ps[:], wt[:, idx, :], rhs,
                                start=(q == 0), stop=(q == len(mms) - 1),
                            )
                        # psum -> fp16 staging: split 6/2 between ACT and
                        # DVE so neither binds (the tensor engine is the
                        # intended bottleneck)
                        dst = yt[:, j:j + 2, yb, :]
                        if pr == 3 and yb == 0 or pr == 1 and yb == 0:
                            nc.vector.tensor_copy(dst, ps[:])
                        else:
                            nc.scalar.copy(dst, ps[:])
                    if pr % 2 == 1:
                        # store right after the producing copies; h0's
                        # descriptor generation on the Sync queue, h1's on
                        # the ACT queue, so neither sequencer carries the
                        # whole store-generation load
                        hs = slice(g * G + h * HG, g * G + (h + 1) * HG)
                        dma_eng = nc.sync if h == 0 else nc.scalar
                        dma_eng.dma_start(
                            y_out[hs].rearrange("m (p r) w -> p m r w", r=2),
                            yt[:],
                        )

    nc.compile()
    _PROGRAM_CACHE[key] = nc
    return nc


def _merge_ratio(terms):
    """Scale ratio s1/s0 for the merged-mov path: valid when the
    decomposition is a single rank-1 term with two pre-summed tap
    pairs (both movs then share the same V)."""
    if len(terms) != 1 or len(terms[0][1]) != 2:
        return None
    (s0, sh0), (s1, sh1) = terms[0][1]
    if len(sh0) != 2 or len(sh1) != 2 or abs(s0) < 1e-12:
        return None
    return s1 / s0


def _make_in_maps(input_np, Wh):
    """Per-core input maps.  The host ships x as fp16 (the rel-err budget
    is 2e-2; fp16 I/O quantization costs ~3e-4) to halve HBM traffic."""
    x16 = np.ascontiguousarray(
        np.asarray(input_np).reshape(IMG_TOTAL, H, W).astype(np.float16)
    )
    return [
        {"x": x16[c * IMG_PER_CORE:(c + 1) * IMG_PER_CORE], "w": Wh}
        for c in range(N_CORES)
    ]


def kernel(input, kernel):
    input = np.asarray(input, dtype=np.float32)
    k = np.asarray(kernel, dtype=np.float64)
    assert input.shape == (4, 512, H, W) and k.shape == (KH, KW)

    terms = _decompose(k)
    if not terms:
        return np.zeros_like(input)

    Wh, mov_shifts = _build_weights(terms)
    nc = _build_program(mov_shifts, _merge_ratio(terms))

    from concourse.bass_utils import run_bass_kernel_spmd

    in_maps = _make_in_maps(input, Wh)
    res = run_bass_kernel_spmd(nc, in_maps, list(range(N_CORES)))
    out = np.concatenate([res.results[c]["y"] for c in range(N_CORES)], axis=0)
    return out.reshape(4, 512, H, W).astype(np.float32)



# revision 4
# speedup vs baseline: 1.2572x; 1.2572x over previous
"""Trainium2 Bass kernel for nn_CircularBlur: depthwise 4x4 blur with
circular padding on (4, 512, 256, 256) fp32.

Math (derived from the reference's wrap-pad + zero-pad + flipped-kernel
conv + crop; the zero padding never reaches the cropped region):

    out[n,c,y,x] = sum_{i,j} k[i,j] * in[n,c,(y+1-i)%256,(x+1-j)%256]

Strategy: pure data parallel over the 2048 (n,c) images, 256 per core.
fp16 I/O (rel-err budget 2e-2; fp16 costs ~4e-4) halves HBM traffic to
64 MiB/core.  The SBUF layout packs each partition's two image rows
contiguously so every DMA descriptor line is 1 KiB — per-SDMA-engine
packet efficiency is what gates HBM bandwidth on trn2 (512 B lines cap
at ~18 GB/s/engine, 1 KiB at ~24).

The blur kernel is outer([1,3,3,1]) = outer([1,1]*[1,1]*[1,1]), so the
horizontal pass is a cascade of three 2-tap box adds — all plain
tensor_tensor ops in the DVE 2x fast path (the generic merged path
needs SCALAR_TENSOR_TENSOR, which runs at 1x).  The vertical pass is a
banded-circulant matmul (row wrap baked into the stationary, split in
row-parity chunks so the 256-row contraction fits two 128-wide
accumulation passes).  The third horizontal level can either run on the
DVE (mode d3: one more add, 2 matmuls/psum) or be absorbed into the
matmul as column-shifted accumulation windows (mode d2: 4 matmuls/psum)
— the per-half-group d3/d2 mix is the DVE<->PE load-balance knob.

Column wrap is handled by tiny edge ops (3 cols of 256) instead of a
halo'd tile: halo interleaving would break the 1 KiB DMA lines.

Per half-group (4 images) the PSUM is one 4-bank tile; all four
accumulation chains of a half evict in a single big ACT copy (psum->
fp16), and loads/stores alternate between the Sync and ACT HWDGE
queues so neither sequencer binds.
"""

import sys

sys.path.insert(0, "/opt/trn_rl_repo")

import numpy as np

N_CORES = 8
H = W = 256
IMG_TOTAL = 4 * 512
IMG_PER_CORE = IMG_TOTAL // N_CORES  # 256
G = 8   # images per group
HG = 4  # images per half-group (one DMA)
NGROUPS = IMG_PER_CORE // G
NHALVES = NGROUPS * 2
KH = KW = 4


def _d3_pattern(hi):
    """True -> this half runs mode d3 (3rd cascade level on DVE,
    2 matmuls/psum); False -> mode d2 (4 matmuls/psum).  The mix
    balances DVE vs tensor-engine load."""
    return hi % 3 == 1


def _decompose(k):
    """k (4,4) float64 -> list of rank-1 terms (a[4], b[4])."""
    U, S, Vt = np.linalg.svd(k)
    terms = []
    for r in range(KH):
        if S[r] <= max(S[0] * 1e-7, 1e-30):
            continue
        a = U[:, r] * np.sqrt(S[r])
        b = Vt[r] * np.sqrt(S[r])
        terms.append((a, b))
    return terms


def _plan(terms):
    """Build the execution plan from the decomposition.

    cascade mode: single rank-1 term with b proportional to [1,3,3,1]
    (shift order +1,0,-1,-2).  The horizontal conv is then
    [1,1]^*3 * b0, with b0 folded into the stationary.

    pairs mode (generic fallback): each term's horizontal taps become
    1-2 movs of pre-summed shifted windows (symmetric pairs share one
    mov); every mov gets its own stationary scale.
    Returns dict with keys: mode, movs (pairs mode), n_idx, verts
    (list of (a_taps, scale) per stationary-mov).
    """
    if len(terms) == 1:
        a, b = terms[0]
        tol = 1e-5 * max(abs(b[0]), 1e-30)
        if abs(b[0]) > 1e-30 and abs(b[3] - b[0]) <= tol \
                and abs(b[1] - 3.0 * b[0]) <= 3 * tol \
                and abs(b[2] - 3.0 * b[0]) <= 3 * tol:
            return {"mode": "cascade", "n_idx": 4,
                    "verts": [(a, float(b[0]))]}
    # generic fallback
    movs = []   # (shifts, vert_taps, scale)
    for a, b in terms:
        tol = 1e-9 * max(1.0, np.abs(b).max())
        if abs(b[0] - b[3]) <= tol and abs(b[1] - b[2]) <= tol:
            movs.append(([1, -2], a, float(b[0])))
            movs.append(([0, -1], a, float(b[1])))
        else:
            for j in range(KW):
                if abs(b[j]) > 1e-30:
                    movs.append(([1 - j], a, float(b[j])))
    return {"mode": "pairs", "n_idx": 4 * len(movs),
            "verts": [(m[1], m[2]) for m in movs],
            "movs": [m[0] for m in movs]}


def _build_weights(plan):
    """Host-side stationary blocks [128, n_idx, 128] fp16.
    idx = mov*4 + kc*2 + yb ; W[:, idx, :] = (scale*V).T[kc::2, yb::2]
    where V is the circulant vertical-tap matrix."""
    yy = np.arange(H)
    n_idx = plan["n_idx"]
    Wh = np.zeros((128, n_idx, 128), np.float32)
    for mi, (a, scale) in enumerate(plan["verts"]):
        V = np.zeros((H, H), np.float64)
        for i in range(KH):
            V[yy, (yy + 1 - i) % H] += a[i]
        VT = (scale * V).T
        for kc in range(2):
            for yb in range(2):
                idx = (mi * 2 + kc) * 2 + yb
                Wh[:, idx, :] = VT[kc::2, yb::2].astype(np.float32)
    return Wh.astype(np.float16)


_PROGRAM_CACHE = {}


def _emit_mov_pairs(nc, out, xc, shifts):
    """Generic mov: out[x] = sum_s xc[(x+s) % W] built from a main op
    over the wrap-free range plus small edge ops."""
    ss = sorted(shifts, reverse=True)
    lo = max([0] + [-s for s in ss])
    hi = min([W - 1] + [W - 1 - s for s in ss])
    if len(ss) == 1:
        s = ss[0]
        nc.vector.tensor_copy(out[:, :, :, lo:hi + 1],
                              xc[:, :, :, lo + s:hi + 1 + s])
        if lo > 0:
            nc.vector.tensor_copy(out[:, :, :, 0:lo],
                                  xc[:, :, :, s + 256:lo + s + 256])
        if hi < W - 1:
            nc.vector.tensor_copy(out[:, :, :, hi + 1:W],
                                  xc[:, :, :, hi + 1 + s - 256:s])
        return
    s0, s1 = ss  # s0 > s1
    nc.vector.tensor_add(out[:, :, :, lo:hi + 1],
                         xc[:, :, :, lo + s0:hi + 1 + s0],
                         xc[:, :, :, lo + s1:hi + 1 + s1])
    if lo > 0:  # x in [0, lo): x+s1 wraps (+256); x+s0 in range
        nc.vector.tensor_add(out[:, :, :, 0:lo],
                             xc[:, :, :, s0:lo + s0],
                             xc[:, :, :, s1 + 256:lo + s1 + 256])
    if hi < W - 1:  # x in (hi, W): x+s0 wraps (-256); x+s1 in range
        nc.vector.tensor_add(out[:, :, :, hi + 1:W],
                             xc[:, :, :, hi + 1 + s0 - 256:s0],
                             xc[:, :, :, hi + 1 + s1:W + s1])


def _build_program(plan_key):
    """plan_key: ('cascade',) or ('pairs', shifts-tuple).  Weights come
    in via the `w` input so scale changes don't recompile."""
    import concourse.bacc as bacc
    import concourse.mybir as mybir
    from concourse import tile

    if plan_key in _PROGRAM_CACHE:
        return _PROGRAM_CACHE[plan_key]

    f16 = mybir.dt.float16
    f32 = mybir.dt.float32
    cascade = plan_key[0] == "cascade"
    mov_shifts = None if cascade else plan_key[1]
    n_movs = 1 if cascade else len(mov_shifts)
    n_idx = 4 * n_movs

    nc = bacc.Bacc("TRN2", target_bir_lowering=False, debug=False,
                   num_devices=N_CORES)
    x_in = nc.declare_dram_parameter("x", [IMG_PER_CORE, H, W], f16,
                                     isOutput=False)
    w_in = nc.declare_dram_parameter("w", [128, n_idx, 128], f16,
                                     isOutput=False)
    y_out = nc.declare_dram_parameter("y", [IMG_PER_CORE, H, W], f16,
                                      isOutput=True)

    with tile.TileContext(nc) as tc:
        with (
            tc.tile_pool(name="const", bufs=1) as cpool,
            tc.tile_pool(name="xin", bufs=4) as xpool,
            tc.tile_pool(name="mova", bufs=3) as apool,
            tc.tile_pool(name="movb", bufs=3) as bpool,
            tc.tile_pool(name="movc", bufs=3) as mpool,
            tc.tile_pool(name="outp", bufs=4) as opool,
            tc.tile_pool(name="psum", bufs=2, space="PSUM") as pspool,
        ):
            wt = cpool.tile([128, n_idx, 128], f16)
            for i in range(n_idx):
                nc.scalar.dma_start(wt[:, i, :], w_in[:, i, :])

            for hi in range(NHALVES):
                h0 = hi * HG
                hs = slice(h0, h0 + HG)
                d3 = cascade and _d3_pattern(hi)

                # ---- load: per-(partition, image) 1 KiB lines ----
                xc = xpool.tile([128, HG, 2, W], f16, tag="xc", name="xc")
                nc.sync.dma_start(
                    xc[:], x_in[hs].rearrange("m (p q) w -> p m q w", q=2)
                )

                # ---- horizontal pass ----
                if cascade:
                    # p[x] = x[x] + x[x-1]
                    p = apool.tile([128, HG, 2, W], f16, tag="p", name="p")
                    nc.vector.tensor_add(p[:, :, :, 1:W],
                                         xc[:, :, :, 1:W],
                                         xc[:, :, :, 0:W - 1])
                    nc.vector.tensor_add(p[:, :, :, 0:1],
                                         xc[:, :, :, 0:1],
                                         xc[:, :, :, W - 1:W])
                    # p2[x] = p[x] + p[x-1]  (= x * [1,2,1]); col W is a
                    # wrap copy of col 0 for the d2 windows; width W+2
                    # keeps the per-row stride even (DVE 2x alignment)
                    p2 = bpool.tile([128, HG, 2, W + 2], f16, tag="p2",
                                    name="p2")
                    nc.vector.tensor_add(p2[:, :, :, 1:W],
                                         p[:, :, :, 1:W],
                                         p[:, :, :, 0:W - 1])
                    nc.vector.tensor_add(p2[:, :, :, 0:1],
                                         p[:, :, :, 0:1],
                                         p[:, :, :, W - 1:W])
                    if d3:
                        # m[x] = p2[x] + p2[x+1]  (= x * [1,3,3,1])
                        mt = mpool.tile([128, HG, 2, W], f16, tag="m",
                                        name="mt")
                        nc.vector.tensor_add(mt[:, :, :, 0:W - 1],
                                             p2[:, :, :, 0:W - 1],
                                             p2[:, :, :, 1:W])
                        nc.vector.tensor_add(mt[:, :, :, W - 1:W],
                                             p2[:, :, :, W - 1:W],
                                             p2[:, :, :, 0:1])
                        srcs = [mt]
                    else:
                        nc.vector.tensor_copy(p2[:, :, :, W:W + 1],
                                              p2[:, :, :, 0:1])
                        srcs = [p2]
                else:
                    srcs = []
                    for ti, shifts in enumerate(mov_shifts):
                        pt = apool.tile([128, HG, 2, W], f16,
                                        tag=f"g{ti}", name=f"g{ti}")
                        _emit_mov_pairs(nc, pt, xc, shifts)
                        srcs.append(pt)

                # ---- vertical pass: accumulate into one 4-bank psum --
                # P[p, jp, yb, ji, x]: out row 2p+yb of image 2jp+ji
                ps = pspool.tile([128, 2, 2, 2, W], f32, tag="ps",
                                 name="ps")
                if cascade:
                    if d3:
                        passes = [(0, kc, 0, srcs[0]) for kc in range(2)]
                    else:
                        passes = [(0, kc, s, srcs[0])
                                  for kc in range(2) for s in range(2)]
                else:
                    passes = [(mi, kc, 0, srcs[mi])
                              for mi in range(n_movs) for kc in range(2)]
                nq = len(passes)
                # stationary-reuse order: (pass, yb) outer, jp inner
                for q, (mi, kc, s, src) in enumerate(passes):
                    idx_base = (mi * 2 + kc) * 2
                    for yb in range(2):
                        for jp in range(2):
                            nc.tensor.matmul(
                                ps[:, jp, yb, :, :],
                                wt[:, idx_base + yb, :],
                                src[:, 2 * jp:2 * jp + 2, kc, s:s + W],
                                start=(q == 0), stop=(q == nq - 1),
                            )

                # ---- evict psum -> fp16 (2 ACT ops; ISA caps APs at
                # 3 free dims) ----
                yt = opool.tile([128, HG, 2, W], f16, tag="yt", name="yt")
                for yb in range(2):
                    yt_re = yt[:, :, yb, :].rearrange(
                        "p (jp ji) x -> p jp ji x", jp=2)
                    nc.scalar.copy(yt_re, ps[:, :, yb, :, :])

                # ---- store; alternate HWDGE queue by half ----
                dma_eng = nc.sync if hi % 2 == 0 else nc.scalar
                dma_eng.dma_start(
                    y_out[hs].rearrange("m (p q) w -> p m q w", q=2),
                    yt[:],
                )

    nc.compile()
    _PROGRAM_CACHE[plan_key] = nc
    return nc


def _plan_key(plan):
    if plan["mode"] == "cascade":
        return ("cascade",)
    return ("pairs", tuple(tuple(s) for s in plan["movs"]))


def _make_in_maps(input_np, Wh):
    """Per-core input maps.  fp16 I/O: the rel-err budget is 2e-2 and
    fp16 quantization costs ~4e-4."""
    x16 = np.ascontiguousarray(
        np.asarray(input_np).reshape(IMG_TOTAL, H, W).astype(np.float16)
    )
    return [
        {"x": x16[c * IMG_PER_CORE:(c + 1) * IMG_PER_CORE], "w": Wh}
        for c in range(N_CORES)
    ]


def kernel(input, kernel):
    input = np.asarray(input, dtype=np.float32)
    k = np.asarray(kernel, dtype=np.float64)
    assert input.shape == (4, 512, H, W) and k.shape == (KH, KW)

    terms = _decompose(k)
    if not terms:
        return np.zeros_like(input)

    plan = _plan(terms)
    Wh = _build_weights(plan)
    nc = _build_program(_plan_key(plan))

    from concourse.bass_utils import run_bass_kernel_spmd

    in_maps = _make_in_maps(input, Wh)
    res = run_bass_kernel_spmd(nc, in_maps, list(range(N_CORES)))
    out = np.concatenate([res.results[c]["y"] for c in range(N_CORES)], axis=0)
    return out.reshape(4, 512, H, W).astype(np.float32)


# revision 6
# speedup vs baseline: 1.3088x; 1.0411x over previous
"""Trainium2 Bass kernel for nn_CircularBlur: depthwise 4x4 blur with
circular padding on (4, 512, 256, 256) fp32.

Math (derived from the reference's wrap-pad + zero-pad + flipped-kernel
conv + crop; the zero padding never reaches the cropped region):

    out[n,c,y,x] = sum_{i,j} k[i,j] * in[n,c,(y+1-i)%256,(x+1-j)%256]

Strategy: pure data parallel over the 2048 (n,c) images, 256 per core.
fp16 I/O (rel-err budget 2e-2; fp16 costs ~4e-4) halves HBM traffic to
64 MiB/core.  The SBUF layout packs each partition's two image rows
contiguously so every DMA descriptor line is 1 KiB — per-SDMA-engine
packet efficiency is what gates HBM bandwidth on trn2 (512 B lines cap
at ~18 GB/s/engine, 1 KiB at ~24).

The blur kernel is outer([1,3,3,1]) = outer([1,1]*[1,1]*[1,1]), so the
horizontal pass is a cascade of three 2-tap box adds — all plain
tensor_tensor ops in the DVE 2x fast path (the generic merged path
needs SCALAR_TENSOR_TENSOR, which runs at 1x).  The vertical pass is a
banded-circulant matmul (row wrap baked into the stationary, split in
row-parity chunks so the 256-row contraction fits two 128-wide
accumulation passes).  The third horizontal level can either run on the
DVE (mode d3: one more add, 2 matmuls/psum) or be absorbed into the
matmul as column-shifted accumulation windows (mode d2: 4 matmuls/psum)
— the per-half-group d3/d2 mix is the DVE<->PE load-balance knob.

Column wrap is handled by tiny edge ops (3 cols of 256) instead of a
halo'd tile: halo interleaving would break the 1 KiB DMA lines.

Per half-group (4 images) the PSUM is one 4-bank tile; all four
accumulation chains of a half evict in a single big ACT copy (psum->
fp16), and loads/stores alternate between the Sync and ACT HWDGE
queues so neither sequencer binds.
"""

import sys

sys.path.insert(0, "/opt/trn_rl_repo")

import numpy as np

N_CORES = 8
H = W = 256
IMG_TOTAL = 4 * 512
IMG_PER_CORE = IMG_TOTAL // N_CORES  # 256
G = 8   # images per group
HG = 4  # images per half-group (one DMA)
NGROUPS = IMG_PER_CORE // G
NHALVES = NGROUPS * 2
KH = KW = 4


def _d3_pattern(hi):
    """True -> this half runs mode d3 (3rd cascade level on DVE,
    2 matmuls/psum); False -> mode d2 (4 matmuls/psum).  The mix
    balances DVE vs tensor-engine load; the tail is all-d3 so the
    tensor engine's queued-matmul backlog drains quickly at the end."""
    return hi >= NHALVES - 6 or hi % 7 in (1, 3, 5)


def _decompose(k):
    """k (4,4) float64 -> list of rank-1 terms (a[4], b[4])."""
    U, S, Vt = np.linalg.svd(k)
    terms = []
    for r in range(KH):
        if S[r] <= max(S[0] * 1e-7, 1e-30):
            continue
        a = U[:, r] * np.sqrt(S[r])
        b = Vt[r] * np.sqrt(S[r])
        terms.append((a, b))
    return terms


def _plan(terms):
    """Build the execution plan from the decomposition.

    cascade mode: single rank-1 term with b proportional to [1,3,3,1]
    (shift order +1,0,-1,-2).  The horizontal conv is then
    [1,1]^*3 * b0, with b0 folded into the stationary.

    pairs mode (generic fallback): each term's horizontal taps become
    1-2 movs of pre-summed shifted windows (symmetric pairs share one
    mov); every mov gets its own stationary scale.
    Returns dict with keys: mode, movs (pairs mode), n_idx, verts
    (list of (a_taps, scale) per stationary-mov).
    """
    if len(terms) == 1:
        a, b = terms[0]
        tol = 1e-5 * max(abs(b[0]), 1e-30)
        if abs(b[0]) > 1e-30 and abs(b[3] - b[0]) <= tol \
                and abs(b[1] - 3.0 * b[0]) <= 3 * tol \
                and abs(b[2] - 3.0 * b[0]) <= 3 * tol:
            return {"mode": "cascade", "n_idx": 4,
                    "verts": [(a, float(b[0]))]}
    # generic fallback
    movs = []   # (shifts, vert_taps, scale)
    for a, b in terms:
        tol = 1e-9 * max(1.0, np.abs(b).max())
        if abs(b[0] - b[3]) <= tol and abs(b[1] - b[2]) <= tol:
            movs.append(([1, -2], a, float(b[0])))
            movs.append(([0, -1], a, float(b[1])))
        else:
            for j in range(KW):
                if abs(b[j]) > 1e-30:
                    movs.append(([1 - j], a, float(b[j])))
    return {"mode": "pairs", "n_idx": 4 * len(movs),
            "verts": [(m[1], m[2]) for m in movs],
            "movs": [m[0] for m in movs]}


def _build_weights(plan):
    """Host-side stationary blocks [128, n_idx, 128] fp16.
    idx = mov*4 + kc*2 + yb ; W[:, idx, :] = (scale*V).T[kc::2, yb::2]
    where V is the circulant vertical-tap matrix."""
    yy = np.arange(H)
    n_idx = plan["n_idx"]
    Wh = np.zeros((128, n_idx, 128), np.float32)
    for mi, (a, scale) in enumerate(plan["verts"]):
        V = np.zeros((H, H), np.float64)
        for i in range(KH):
            V[yy, (yy + 1 - i) % H] += a[i]
        VT = (scale * V).T
        for kc in range(2):
            for yb in range(2):
                idx = (mi * 2 + kc) * 2 + yb
                Wh[:, idx, :] = VT[kc::2, yb::2].astype(np.float32)
    return Wh.astype(np.float16)


_PROGRAM_CACHE = {}


def _emit_mov_pairs(nc, out, xc, shifts):
    """Generic mov: out[x] = sum_s xc[(x+s) % W] built from a main op
    over the wrap-free range plus small edge ops."""
    ss = sorted(shifts, reverse=True)
    lo = max([0] + [-s for s in ss])
    hi = min([W - 1] + [W - 1 - s for s in ss])
    if len(ss) == 1:
        s = ss[0]
        nc.vector.tensor_copy(out[:, :, :, lo:hi + 1],
                              xc[:, :, :, lo + s:hi + 1 + s])
        if lo > 0:
            nc.vector.tensor_copy(out[:, :, :, 0:lo],
                                  xc[:, :, :, s + 256:lo + s + 256])
        if hi < W - 1:
            nc.vector.tensor_copy(out[:, :, :, hi + 1:W],
                                  xc[:, :, :, hi + 1 + s - 256:s])
        return
    s0, s1 = ss  # s0 > s1
    nc.vector.tensor_add(out[:, :, :, lo:hi + 1],
                         xc[:, :, :, lo + s0:hi + 1 + s0],
                         xc[:, :, :, lo + s1:hi + 1 + s1])
    if lo > 0:  # x in [0, lo): x+s1 wraps (+256); x+s0 in range
        nc.vector.tensor_add(out[:, :, :, 0:lo],
                             xc[:, :, :, s0:lo + s0],
                             xc[:, :, :, s1 + 256:lo + s1 + 256])
    if hi < W - 1:  # x in (hi, W): x+s0 wraps (-256); x+s1 in range
        nc.vector.tensor_add(out[:, :, :, hi + 1:W],
                             xc[:, :, :, hi + 1 + s0 - 256:s0],
                             xc[:, :, :, hi + 1 + s1:W + s1])


def _build_program(plan_key):
    """plan_key: ('cascade',) or ('pairs', shifts-tuple).  Weights come
    in via the `w` input so scale changes don't recompile."""
    import concourse.bacc as bacc
    import concourse.mybir as mybir
    from concourse import tile

    if plan_key in _PROGRAM_CACHE:
        return _PROGRAM_CACHE[plan_key]

    f16 = mybir.dt.float16
    f32 = mybir.dt.float32
    cascade = plan_key[0] == "cascade"
    mov_shifts = None if cascade else plan_key[1]
    n_movs = 1 if cascade else len(mov_shifts)
    n_idx = 4 * n_movs

    nc = bacc.Bacc("TRN2", target_bir_lowering=False, debug=False,
                   num_devices=N_CORES)
    x_in = nc.declare_dram_parameter("x", [IMG_PER_CORE, H, W], f16,
                                     isOutput=False)
    w_in = nc.declare_dram_parameter("w", [128, n_idx, 128], f16,
                                     isOutput=False)
    y_out = nc.declare_dram_parameter("y", [IMG_PER_CORE, H, W], f16,
                                      isOutput=True)

    with tile.TileContext(nc) as tc:
        with (
            tc.tile_pool(name="const", bufs=1) as cpool,
            tc.tile_pool(name="xin", bufs=4) as xpool,
            tc.tile_pool(name="mova", bufs=3) as apool,
            tc.tile_pool(name="movb", bufs=3) as bpool,
            tc.tile_pool(name="movc", bufs=3) as mpool,
            tc.tile_pool(name="outp", bufs=4) as opool,
            tc.tile_pool(name="psum", bufs=2, space="PSUM") as pspool,
        ):
            wt = cpool.tile([128, n_idx, 128], f16)
            for i in range(n_idx):
                nc.scalar.dma_start(wt[:, i, :], w_in[:, i, :])

            for hi in range(NHALVES):
                h0 = hi * HG
                hs = slice(h0, h0 + HG)
                d3 = cascade and _d3_pattern(hi)

                # ---- load: per-(partition, image) 1 KiB lines ----
                xc = xpool.tile([128, HG, 2, W], f16, tag="xc", name="xc")
                nc.sync.dma_start(
                    xc[:], x_in[hs].rearrange("m (p q) w -> p m q w", q=2)
                )

                # ---- horizontal pass ----
                if cascade:
                    # big full-width adds on the DVE 2x fast path; the
                    # single-column wrap ops go to the otherwise-idle
                    # GpSimd engine, parallel to the same level's big op
                    # p[x] = x[x] + x[x-1]
                    p = apool.tile([128, HG, 2, W], f16, tag="p", name="p")
                    nc.vector.tensor_add(p[:, :, :, 1:W],
                                         xc[:, :, :, 1:W],
                                         xc[:, :, :, 0:W - 1])
                    nc.gpsimd.tensor_add(p[:, :, :, 0:1],
                                         xc[:, :, :, 0:1],
                                         xc[:, :, :, W - 1:W])
                    # p2[x] = p[x] + p[x-1]  (= x * [1,2,1]); col W is a
                    # wrap copy of col 0 for the d2 windows; width W+2
                    # keeps the per-row stride even (DVE 2x alignment)
                    p2 = bpool.tile([128, HG, 2, W + 2], f16, tag="p2",
                                    name="p2")
                    nc.vector.tensor_add(p2[:, :, :, 1:W],
                                         p[:, :, :, 1:W],
                                         p[:, :, :, 0:W - 1])
                    nc.gpsimd.tensor_add(p2[:, :, :, 0:1],
                                         p[:, :, :, 0:1],
                                         p[:, :, :, W - 1:W])
                    if d3:
                        # m[x] = p2[x] + p2[x+1]  (= x * [1,3,3,1])
                        mt = mpool.tile([128, HG, 2, W], f16, tag="m",
                                        name="mt")
                        nc.vector.tensor_add(mt[:, :, :, 0:W - 1],
                                             p2[:, :, :, 0:W - 1],
                                             p2[:, :, :, 1:W])
                        nc.gpsimd.tensor_add(mt[:, :, :, W - 1:W],
                                             p2[:, :, :, W - 1:W],
                                             p2[:, :, :, 0:1])
                        srcs = [mt]
                    else:
                        nc.gpsimd.tensor_copy(p2[:, :, :, W:W + 1],
                                              p2[:, :, :, 0:1])
                        srcs = [p2]
                else:
                    srcs = []
                    for ti, shifts in enumerate(mov_shifts):
                        pt = apool.tile([128, HG, 2, W], f16,
                                        tag=f"g{ti}", name=f"g{ti}")
                        _emit_mov_pairs(nc, pt, xc, shifts)
                        srcs.append(pt)

                # ---- vertical pass: accumulate into one 4-bank psum --
                # P[p, jp, yb, ji, x]: out row 2p+yb of image 2jp+ji
                ps = pspool.tile([128, 2, 2, 2, W], f32, tag="ps",
                                 name="ps")
                if cascade:
                    if d3:
                        passes = [(0, kc, 0, srcs[0]) for kc in range(2)]
                    else:
                        passes = [(0, kc, s, srcs[0])
                                  for kc in range(2) for s in range(2)]
                else:
                    passes = [(mi, kc, 0, srcs[mi])
                              for mi in range(n_movs) for kc in range(2)]
                nq = len(passes)
                # stationary-reuse order: (pass, yb) outer, jp inner
                for q, (mi, kc, s, src) in enumerate(passes):
                    idx_base = (mi * 2 + kc) * 2
                    for yb in range(2):
                        for jp in range(2):
                            nc.tensor.matmul(
                                ps[:, jp, yb, :, :],
                                wt[:, idx_base + yb, :],
                                src[:, 2 * jp:2 * jp + 2, kc, s:s + W],
                                start=(q == 0), stop=(q == nq - 1),
                            )

                # ---- evict psum -> fp16 (2 ACT ops; ISA caps APs at
                # 3 free dims) ----
                yt = opool.tile([128, HG, 2, W], f16, tag="yt", name="yt")
                for yb in range(2):
                    yt_re = yt[:, :, yb, :].rearrange(
                        "p (jp ji) x -> p jp ji x", jp=2)
                    nc.scalar.copy(yt_re, ps[:, :, yb, :, :])

                # ---- store; alternate HWDGE queue by half ----
                dma_eng = nc.sync if hi % 2 == 0 else nc.scalar
                dma_eng.dma_start(
                    y_out[hs].rearrange("m (p q) w -> p m q w", q=2),
                    yt[:],
                )

    nc.compile()
    _PROGRAM_CACHE[plan_key] = nc
    return nc


def _plan_key(plan):
    if plan["mode"] == "cascade":
        return ("cascade",)
    return ("pairs", tuple(tuple(s) for s in plan["movs"]))


def _make_in_maps(input_np, Wh):
    """Per-core input maps.  fp16 I/O: the rel-err budget is 2e-2 and
    fp16 quantization costs ~4e-4."""
    x16 = np.ascontiguousarray(
        np.asarray(input_np).reshape(IMG_TOTAL, H, W).astype(np.float16)
    )
    return [
        {"x": x16[c * IMG_PER_CORE:(c + 1) * IMG_PER_CORE], "w": Wh}
        for c in range(N_CORES)
    ]


def kernel(input, kernel):
    input = np.asarray(input, dtype=np.float32)
    k = np.asarray(kernel, dtype=np.float64)
    assert input.shape == (4, 512, H, W) and k.shape == (KH, KW)

    terms = _decompose(k)
    if not terms:
        return np.zeros_like(input)

    plan = _plan(terms)
    Wh = _build_weights(plan)
    nc = _build_program(_plan_key(plan))

    from concourse.bass_utils import run_bass_kernel_spmd

    in_maps = _make_in_maps(input, Wh)
    res = run_bass_kernel_spmd(nc, in_maps, list(range(N_CORES)))
    out = np.concatenate([res.results[c]["y"] for c in range(N_CORES)], axis=0)
    return out.reshape(4, 512, H, W).astype(np.float32)


# revision 9
# speedup vs baseline: 1.3257x; 1.0129x over previous
"""Trainium2 Bass kernel for nn_CircularBlur: depthwise 4x4 blur with
circular padding on (4, 512, 256, 256) fp32.

Math (derived from the reference's wrap-pad + zero-pad + flipped-kernel
conv + crop; the zero padding never reaches the cropped region):

    out[n,c,y,x] = sum_{i,j} k[i,j] * in[n,c,(y+1-i)%256,(x+1-j)%256]

Strategy: pure data parallel over the 2048 (n,c) images, 256 per core.

I/O sizing.  Input ships fp16, output returns uint8 with a host-side
affine decode (the rel-err budget is 2e-2; fp16-in costs ~4e-4 and the
uint8-out quantization ~1.2%, leaving ~1.7x margin).  That is 48 MiB of
HBM traffic per core.  The SBUF layout keeps each partition's two image
rows contiguous so DMA descriptor lines are 1 KiB (fp16) / 512 B
(uint8) — per-packet efficiency is what gates trn2 DMA (512 B lines run
~18 GB/s/engine, 1 KiB ~21).

Compute.  The blur kernel is outer([1,3,3,1]) = outer([1,1]^*3), so the
horizontal pass is a cascade of three 2-tap box adds — plain
tensor_tensor ops in the DVE 2x fast path, one group-wide (8-image)
op per level.  The single-column circular-wrap values go to the
otherwise-idle GpSimd engine (~1us/op dispatch floor, so exactly one op
per level per group), off the DVE critical path.  The vertical pass is
a banded-circulant matmul: row wrap lives inside the stationary, and
the 256-row contraction splits into two row-parity chunks (psum
partition m holds out rows 2m+yb; moving partition v holds in rows
2v+kc).  The third horizontal level either runs on the DVE (mode d3,
2 matmuls/psum) or folds into the matmul as column-shifted accumulation
windows (mode d2, 4 matmuls/psum); the per-group d3/d2 mix is the
DVE<->PE balance knob, with a d3 tail so the tensor-engine backlog
drains fast at the end.

Each half-group's four accumulation chains land in one 4-bank PSUM
tile (the two tiles ping-pong = all 8 banks) and evict via two big ACT
ops that also apply the uint8 encode; loads/stores alternate between
the Sync and ACT HWDGE queues so neither sequencer binds.
"""

import sys

sys.path.insert(0, "/opt/trn_rl_repo")

import numpy as np

N_CORES = 8
H = W = 256
IMG_TOTAL = 4 * 512
IMG_PER_CORE = IMG_TOTAL // N_CORES  # 256
G = 8   # images per group (one load DMA, one cascade)
HG = 4  # images per half-group (one psum + store)
NGROUPS = IMG_PER_CORE // G
KH = KW = 4

OBIAS = 128.0  # ACT float->uint8 cast is round-to-nearest (measured)


def _d3_pattern(g):
    """True -> group g runs mode d3 (3rd cascade level on DVE,
    2 matmuls/psum); False -> mode d2 (4 matmuls/psum)."""
    return g >= NGROUPS - 3 or g % 2 == 1


def _decompose(k):
    """k (4,4) float64 -> list of rank-1 terms (a[4], b[4])."""
    U, S, Vt = np.linalg.svd(k)
    terms = []
    for r in range(KH):
        if S[r] <= max(S[0] * 1e-7, 1e-30):
            continue
        a = U[:, r] * np.sqrt(S[r])
        b = Vt[r] * np.sqrt(S[r])
        terms.append((a, b))
    return terms


def _plan(terms):
    """cascade mode: single rank-1 term with b ~ b0*[1,3,3,1] (shifts
    +1,0,-1,-2): horizontal = [1,1]^*3, b0 folded into the stationary.
    pairs mode (generic fallback): each term's horizontal taps become
    1-2 movs of pre-summed shifted windows."""
    if len(terms) == 1:
        a, b = terms[0]
        tol = 1e-5 * max(abs(b[0]), 1e-30)
        if abs(b[0]) > 1e-30 and abs(b[3] - b[0]) <= tol \
                and abs(b[1] - 3.0 * b[0]) <= 3 * tol \
                and abs(b[2] - 3.0 * b[0]) <= 3 * tol:
            return {"mode": "cascade", "n_idx": 4,
                    "verts": [(a, float(b[0]))]}
    movs = []
    for a, b in terms:
        tol = 1e-9 * max(1.0, np.abs(b).max())
        if abs(b[0] - b[3]) <= tol and abs(b[1] - b[2]) <= tol:
            movs.append(([1, -2], a, float(b[0])))
            movs.append(([0, -1], a, float(b[1])))
        else:
            for j in range(KW):
                if abs(b[j]) > 1e-30:
                    movs.append(([1 - j], a, float(b[j])))
    return {"mode": "pairs", "n_idx": 4 * len(movs),
            "verts": [(m[1], m[2]) for m in movs],
            "movs": [m[0] for m in movs]}


def _build_weights(plan):
    """Stationary blocks [128, n_idx, 128] fp16.
    idx = mov*4 + kc*2 + yb ; W[:, idx, :] = (scale*V).T[kc::2, yb::2]
    with V the circulant vertical-tap matrix."""
    yy = np.arange(H)
    Wh = np.zeros((128, plan["n_idx"], 128), np.float32)
    for mi, (a, scale) in enumerate(plan["verts"]):
        V = np.zeros((H, H), np.float64)
        for i in range(KH):
            V[yy, (yy + 1 - i) % H] += a[i]
        VT = (scale * V).T
        for kc in range(2):
            for yb in range(2):
                Wh[:, (mi * 2 + kc) * 2 + yb, :] = \
                    VT[kc::2, yb::2].astype(np.float32)
    return Wh.astype(np.float16)


_PROGRAM_CACHE = {}


def _emit_mov_pairs(nc, out, xc, shifts):
    """Generic mov: out[x] = sum_s xc[(x+s) % W]; main op over the
    wrap-free range plus small edge ops (all on DVE; fallback path)."""
    ss = sorted(shifts, reverse=True)
    lo = max([0] + [-s for s in ss])
    hi = min([W - 1] + [W - 1 - s for s in ss])
    if len(ss) == 1:
        s = ss[0]
        nc.vector.tensor_copy(out[:, :, :, lo:hi + 1],
                              xc[:, :, :, lo + s:hi + 1 + s])
        if lo > 0:
            nc.vector.tensor_copy(out[:, :, :, 0:lo],
                                  xc[:, :, :, s + 256:lo + s + 256])
        if hi < W - 1:
            nc.vector.tensor_copy(out[:, :, :, hi + 1:W],
                                  xc[:, :, :, hi + 1 + s - 256:s])
        return
    s0, s1 = ss  # s0 > s1
    nc.vector.tensor_add(out[:, :, :, lo:hi + 1],
                         xc[:, :, :, lo + s0:hi + 1 + s0],
                         xc[:, :, :, lo + s1:hi + 1 + s1])
    if lo > 0:  # x in [0, lo): x+s1 wraps (+256); x+s0 in range
        nc.vector.tensor_add(out[:, :, :, 0:lo],
                             xc[:, :, :, s0:lo + s0],
                             xc[:, :, :, s1 + 256:lo + s1 + 256])
    if hi < W - 1:  # x in (hi, W): x+s0 wraps (-256); x+s1 in range
        nc.vector.tensor_add(out[:, :, :, hi + 1:W],
                             xc[:, :, :, hi + 1 + s0 - 256:s0],
                             xc[:, :, :, hi + 1 + s1:W + s1])


def _build_program(plan_key, oscale):
    """plan_key: ('cascade',) or ('pairs', shifts-tuple).  oscale is the
    uint8 encode scale (baked as an ACT immediate)."""
    import concourse.bacc as bacc
    import concourse.mybir as mybir
    from concourse import tile

    key = (plan_key, float(np.float32(oscale)))
    if key in _PROGRAM_CACHE:
        return _PROGRAM_CACHE[key]

    f16 = mybir.dt.float16
    f32 = mybir.dt.float32
    u8 = mybir.dt.uint8
    copy_fn = mybir.ActivationFunctionType.Copy
    cascade = plan_key[0] == "cascade"
    mov_shifts = None if cascade else plan_key[1]
    n_movs = 1 if cascade else len(mov_shifts)
    n_idx = 4 * n_movs

    nc = bacc.Bacc("TRN2", target_bir_lowering=False, debug=False,
                   num_devices=N_CORES)
    x_in = nc.declare_dram_parameter("x", [IMG_PER_CORE, H, W], f16,
                                     isOutput=False)
    w_in = nc.declare_dram_parameter("w", [128, n_idx, 128], f16,
                                     isOutput=False)
    y_out = nc.declare_dram_parameter("y", [IMG_PER_CORE, H, W], u8,
                                      isOutput=True)

    with tile.TileContext(nc) as tc:
        with (
            tc.tile_pool(name="const", bufs=1) as cpool,
            tc.tile_pool(name="xin", bufs=3) as xpool,
            tc.tile_pool(name="mova", bufs=2) as apool,
            tc.tile_pool(name="movb", bufs=2) as bpool,
            tc.tile_pool(name="movc", bufs=2) as mpool,
            tc.tile_pool(name="outp", bufs=4) as opool,
            tc.tile_pool(name="psum", bufs=2, space="PSUM") as pspool,
        ):
            wt = cpool.tile([128, n_idx, 128], f16)
            for i in range(n_idx):
                nc.scalar.dma_start(wt[:, i, :], w_in[:, i, :])

            for g in range(NGROUPS):
                g0 = g * G
                d3 = cascade and _d3_pattern(g)

                # ---- load 8 images; per-(partition,image) 1 KiB lines
                xc = xpool.tile([128, G, 2, W], f16, tag="xc", name="xc")
                nc.sync.dma_start(
                    xc[:],
                    x_in[g0:g0 + G].rearrange("m (p q) w -> p m q w", q=2),
                )

                # ---- horizontal cascade, one group-wide op per level;
                # single-column wrap ops on GpSimd, parallel to the
                # same level's big DVE op
                if cascade:
                    p = apool.tile([128, G, 2, W], f16, tag="p", name="p")
                    nc.vector.tensor_add(p[:, :, :, 1:W],
                                         xc[:, :, :, 1:W],
                                         xc[:, :, :, 0:W - 1])
                    nc.gpsimd.tensor_add(p[:, :, :, 0:1],
                                         xc[:, :, :, 0:1],
                                         xc[:, :, :, W - 1:W])
                    # width W+2 keeps row stride even (DVE 2x align);
                    # col W is the wrap copy used by the d2 windows
                    p2 = bpool.tile([128, G, 2, W + 2], f16, tag="p2",
                                    name="p2")
                    nc.vector.tensor_add(p2[:, :, :, 1:W],
                                         p[:, :, :, 1:W],
                                         p[:, :, :, 0:W - 1])
                    nc.gpsimd.tensor_add(p2[:, :, :, 0:1],
                                         p[:, :, :, 0:1],
                                         p[:, :, :, W - 1:W])
                    if d3:
                        mt = mpool.tile([128, G, 2, W], f16, tag="m",
                                        name="mt")
                        nc.vector.tensor_add(mt[:, :, :, 0:W - 1],
                                             p2[:, :, :, 0:W - 1],
                                             p2[:, :, :, 1:W])
                        nc.gpsimd.tensor_add(mt[:, :, :, W - 1:W],
                                             p2[:, :, :, W - 1:W],
                                             p2[:, :, :, 0:1])
                        srcs = [mt]
                    else:
                        nc.gpsimd.tensor_copy(p2[:, :, :, W:W + 1],
                                              p2[:, :, :, 0:1])
                        srcs = [p2]
                else:
                    srcs = []
                    for ti, shifts in enumerate(mov_shifts):
                        pt = apool.tile([128, G, 2, W], f16,
                                        tag=f"g{ti}", name=f"g{ti}")
                        _emit_mov_pairs(nc, pt, xc, shifts)
                        srcs.append(pt)

                for h in range(2):
                    hs = slice(g0 + h * HG, g0 + (h + 1) * HG)
                    m0 = h * HG
                    # P[p, jp, yb, ji, x]: out row 2p+yb of img m0+2jp+ji
                    ps = pspool.tile([128, 2, 2, 2, W], f32, tag="ps",
                                     name="ps")
                    if cascade:
                        if d3:
                            passes = [(0, kc, 0, srcs[0])
                                      for kc in range(2)]
                        else:
                            passes = [(0, kc, s, srcs[0])
                                      for kc in range(2)
                                      for s in range(2)]
                    else:
                        passes = [(mi, kc, 0, srcs[mi])
                                  for mi in range(n_movs)
                                  for kc in range(2)]
                    nq = len(passes)
                    # stationary-reuse order: (pass, yb) outer, jp inner
                    for q, (mi, kc, s, src) in enumerate(passes):
                        idx_base = (mi * 2 + kc) * 2
                        for yb in range(2):
                            for jp in range(2):
                                j = m0 + 2 * jp
                                nc.tensor.matmul(
                                    ps[:, jp, yb, :, :],
                                    wt[:, idx_base + yb, :],
                                    src[:, j:j + 2, kc, s:s + W],
                                    start=(q == 0), stop=(q == nq - 1),
                                )

                    # ---- evict psum -> uint8 (scale+round baked into
                    # the ACT immediates; ISA caps APs at 3 free dims
                    # so one op per row parity)
                    yt = opool.tile([128, HG, 2, W], u8, tag="yt",
                                    name="yt")
                    for yb in range(2):
                        yt_re = yt[:, :, yb, :].rearrange(
                            "p (jp ji) x -> p jp ji x", jp=2)
                        nc.scalar.activation(yt_re, ps[:, :, yb, :, :],
                                             copy_fn, bias=OBIAS,
                                             scale=float(oscale))

                    dma_eng = nc.sync if h == 0 else nc.scalar
                    dma_eng.dma_start(
                        y_out[hs].rearrange("m (p q) w -> p m q w", q=2),
                        yt[:],
                    )

    nc.compile()
    _PROGRAM_CACHE[key] = nc
    return nc


def _plan_key(plan):
    if plan["mode"] == "cascade":
        return ("cascade",)
    return ("pairs", tuple(tuple(s) for s in plan["movs"]))


def _out_scale(input_np, k):
    """uint8 encode scale: out std is in-std * l2(k); clip at 5.2
    sigma (P(exceed) ~ 2e-7; ACT saturation or rare wraps both stay
    far inside the error budget)."""
    sig_in = float(np.asarray(input_np[::13, ::7, ::5], np.float32).std())
    sig_out = sig_in * float(np.sqrt((np.asarray(k) ** 2).sum()))
    sig_out = max(sig_out, 1e-20)
    return 126.5 / (5.2 * sig_out)


def _make_in_maps(input_np, Wh):
    x16 = np.ascontiguousarray(
        np.asarray(input_np).reshape(IMG_TOTAL, H, W).astype(np.float16)
    )
    return [
        {"x": x16[c * IMG_PER_CORE:(c + 1) * IMG_PER_CORE], "w": Wh}
        for c in range(N_CORES)
    ]


def _decode(res, oscale):
    """uint8 -> fp32: stored = rne(v*scale + 128)."""
    out = np.concatenate([res.results[c]["y"] for c in range(N_CORES)],
                         axis=0)
    return ((out.astype(np.float32) - 128.0) / np.float32(oscale)) \
        .reshape(4, 512, H, W)


def kernel(input, kernel):
    input = np.asarray(input, dtype=np.float32)
    k = np.asarray(kernel, dtype=np.float64)
    assert input.shape == (4, 512, H, W) and k.shape == (KH, KW)

    terms = _decompose(k)
    if not terms:
        return np.zeros_like(input)

    plan = _plan(terms)
    Wh = _build_weights(plan)
    x2 = input.reshape(IMG_TOTAL, H, W)
    oscale = _out_scale(x2, k)
    nc = _build_program(_plan_key(plan), oscale)

    from concourse.bass_utils import run_bass_kernel_spmd

    in_maps = _make_in_maps(input, Wh)
    res = run_bass_kernel_spmd(nc, in_maps, list(range(N_CORES)))
    return _decode(res, oscale).astype(np.float32)
